# revision 8
# baseline (speedup 1.0000x reference)
"""Bahdanau-attention score kernel (softmax(v . tanh(W[h;enc]+b))) for 8 TRN2 cores.

v10: bf16 GEMM inputs (halves HBM traffic: enc 19.8MB->9.9MB/core, W
2.2->1.1MB; empirically 9.0e-3 max rel err vs the 2e-2 gate, dominated
by input quantization -- hb stays f32 on host, one-hot rows exact in
bf16), PE warm-up matmuls during the DMA prime window so the HAM clock
gate promotes to 2.4GHz before tile 0 (v9 lost ~10us to a 1.2GHz cold
window after an 8.8us DMA stall demoted it), 3-deep enc tile pool for
true 2-batch prefetch without burst stalls, per-batch softmax phase-1
(16 score cols each, emitted mid-next-batch so the in-order PE queue
never waits) instead of v9's two 32-col halves (the second of which
serialized the entire tail), and the v9 staples: host pre-transposed
enc, host-precomputed h_proj+b folded in via one-hot contraction rows,
DVE mul with the free-axis reduce alternating DVE/ACT, block-ones
matmul softmax denominator, bank-major emission on the final tile.

Self-contained: hardcodes shapes B=32, S=2048, ENC2=600, DD=900.
Sharding: data-parallel over batch (4 batches/core), weights replicated.
"""

import numpy as np
import ml_dtypes

import concourse.bass as bass  # noqa: F401
import concourse.mybir as mybir
import concourse.tile as tile
from concourse import bacc
from concourse.bass_utils import run_bass_kernel_spmd

F32 = mybir.dt.float32
F32R = mybir.dt.float32r
BF16 = mybir.dt.bfloat16
NP_BF16 = ml_dtypes.bfloat16
AF = mybir.ActivationFunctionType
ALU = mybir.AluOpType
AX = mybir.AxisListType

NCORES = 8
B, S, E2, DD = 32, 2048, 600, 900
IN_DIM = DD + E2            # 1500
BL = B // NCORES            # 4 batches per core
SROWS = BL * S              # 8192 s-rows per core
P = 128
NT = S // P                 # 16 s-tiles per batch
NCOL = SROWS // P           # 64 score columns
KA = 92                     # chunk-4 contraction: 88 e-rows + 4 one-hot rows
NSP = [(0, 512), (512, 388)]  # N splits of 900 (PSUM bank = 512 f32)
NWARM = 16                  # HAM warm-up matmuls during the DMA prime window


def build():
    nc = bacc.Bacc("TRN2", target_bir_lowering=False)
    encm_ext = nc.dram_tensor("encm", [512, SROWS], BF16, kind="ExternalInput")
    enc4_ext = nc.dram_tensor("enc4", [BL * KA, S], BF16, kind="ExternalInput")
    rhsm_ext = nc.dram_tensor("rhsm", [512, DD], BF16, kind="ExternalInput")
    rhs4_ext = nc.dram_tensor("rhs4", [KA, DD], BF16, kind="ExternalInput")
    v_ext = nc.dram_tensor("v", [1, DD], F32R, kind="ExternalInput")
    ones_ext = nc.dram_tensor("ones", [1, P], F32R, kind="ExternalInput")
    bones_ext = nc.dram_tensor("bones", [NT, NT], F32, kind="ExternalInput")
    ident_ext = nc.dram_tensor("ident", [P, P], F32, kind="ExternalInput")
    out_ext = nc.dram_tensor("out", [BL, S], F32, kind="ExternalOutput")

    with tile.TileContext(nc) as tc:
        with (
            tc.tile_pool(name="stat", bufs=1) as stat,
            tc.tile_pool(name="encp", bufs=3) as encp,
            tc.tile_pool(name="zp", bufs=4) as zp,
            tc.tile_pool(name="jp", bufs=3) as jp,
            tc.tile_pool(name="ps_e", bufs=4, space="PSUM") as ps_e,
        ):
            # tiny v/ones first (feed the PE warm-up block), then the
            # critical stream: rhs chunk c interleaved with batch-0 enc
            # chunk c (halves, so tile 0 gates on 5 half-tile DMAs not
            # 5 full ones).
            v_row = stat.tile([1, DD], F32R)
            nc.sync.dma_start(out=v_row[:, :], in_=v_ext.ap())
            ones_t = stat.tile([1, P], F32R)
            nc.sync.dma_start(out=ones_t[:, :], in_=ones_ext.ap())

            rhs = []
            cm_tiles = {}
            b0 = []
            bones = stat.tile([NT, NT], F32)
            ident_f = stat.tile([P, P], F32)
            # three parallel dispatch queues for the head: rhs weights on
            # Sync, batch-0 enc first halves on Scalar, second halves on
            # GpSimd -- each DMA_DIRECT2D dispatch costs ~650ns serially
            # per engine queue, so one queue alone gates tile 0 by ~10us
            for c in range(5):
                kp = P if c < 4 else KA
                r = stat.tile([kp, DD], BF16, name=f"rhs{c}")
                if c < 4:
                    nc.sync.dma_start(
                        out=r[:, :], in_=rhsm_ext.ap()[c * P:(c + 1) * P, :]
                    )
                else:
                    nc.sync.dma_start(out=r[:, :], in_=rhs4_ext.ap())
                rhs.append(r)
                t_ = encp.tile([kp, S], BF16, tag=f"cm{c}", name=f"cm{c}_0")
                src = (encm_ext.ap()[c * P:(c + 1) * P, 0:S] if c < 4
                       else enc4_ext.ap()[0:KA, :])
                nc.scalar.dma_start(out=t_[:, 0:8 * P], in_=src[:, 0:8 * P])
                b0.append((t_, src))
            nc.sync.dma_start(out=bones[:, :], in_=bones_ext.ap())
            for (t_, src) in b0:
                nc.gpsimd.dma_start(out=t_[:, 8 * P:S], in_=src[:, 8 * P:S])
            nc.sync.dma_start(out=ident_f[:, :], in_=ident_ext.ap())
            cm_tiles[0] = [t_ for (t_, _) in b0]

            # v_rep = ones^T @ v, then NWARM dummy matmuls into the same
            # PSUM slot: ~2.5us of PE busy while the enc stream lands, so
            # the HAM activity window promotes the PE clock 4/8 -> 8/8
            # before tile 0 instead of ~10us into the main loop.
            v_rep = stat.tile([P, DD], F32)
            psv = ps_e.tile([P, DD], F32, tag="ep", name="ps_vrep")
            for (no, nn) in NSP:
                nc.tensor.matmul(psv[:, no:no + nn], ones_t[0:1, :],
                                 v_row[0:1, no:no + nn],
                                 start=True, stop=True)
            nc.scalar.copy(v_rep[:, :], psv[:, :])
            for _ in range(NWARM):
                nc.tensor.matmul(psv[:, 0:512], ones_t[0:1, :],
                                 v_row[0:1, 0:512], start=True, stop=True)

            def issue_batch(b):
                tiles = []
                for c in range(5):
                    kp = P if c < 4 else KA
                    t_ = encp.tile([kp, S], BF16, tag=f"cm{c}", name=f"cm{c}_{b}")
                    src = (encm_ext.ap()[c * P:(c + 1) * P, b * S:(b + 1) * S]
                           if c < 4 else enc4_ext.ap()[b * KA:(b + 1) * KA, :])
                    nc.sync.dma_start(out=t_[:, :], in_=src)
                    tiles.append(t_)
                cm_tiles[b] = tiles

            issue_batch(1)

            scores = [stat.tile([P, NT], F32, name=f"scores{h}")
                      for h in range(BL)]
            sc_ab = stat.tile([P, 2], F32)
            e1 = [stat.tile([NT, P], F32, name=f"e1_{h}") for h in range(BL)]
            rs = [stat.tile([NT, 1], F32, name=f"rs{h}") for h in range(BL)]
            rfac = [stat.tile([NT, 1], F32, name=f"rfac{h}") for h in range(BL)]
            outf = [stat.tile([NT, P], F32, name=f"outf{h}") for h in range(BL)]
            dve_scr = stat.tile([1, 4], F32)

            # engine primes: absorb DMA sems before the hot loop
            nc.vector.tensor_copy(out=dve_scr[0:1, 0:1], in_=v_rep[0:1, 0:1])
            nc.vector.tensor_copy(out=dve_scr[0:1, 1:2], in_=bones[0:1, 0:1])

            # ---------------- main loop ----------------
            def softmax_a(h):
                # transpose + exp one batch's 16 scores columns; the
                # per-batch scores tile was last written a full batch ago
                # so the PE transpose never waits
                pss = ps_e.tile([P, DD], F32, tag="ep", name=f"ps_sm{h}")
                nc.tensor.transpose(pss[0:NT, 0:P],
                                    scores[h][:, :],
                                    ident_f[:, :])
                nc.scalar.activation(
                    e1[h][:, :], pss[0:NT, 0:P], AF.Exp,
                    accum_out=rs[h][:, :],
                )

            def softmax_b(h):
                # denominator + scale + out DMA; emitted several tiles
                # after softmax_a so the psd matmul finds rs ready and
                # never blocks the in-order PE queue
                # bones is all-ones: psd = sum(rs[h]) replicated over 16
                # partitions
                psd = ps_e.tile([P, DD], F32, tag="ep", name=f"ps_bs{h}")
                nc.tensor.matmul(psd[0:NT, 0:1], bones[0:NT, 0:NT],
                                 rs[h][:, :], start=True, stop=True)
                nc.vector.reciprocal(rfac[h][:, :], psd[0:NT, 0:1])
                nc.vector.tensor_scalar_mul(outf[h][:, :], e1[h][:, :],
                                            rfac[h][:, 0:1])
                nc.sync.dma_start(
                    out=out_ext.ap()[h:h + 1, :].rearrange(
                        "b (t p) -> (b t) p", p=P),
                    in_=outf[h][:, :],
                )

            for b in range(BL):
                if b + 2 < BL:
                    issue_batch(b + 2)
                cm = cm_tiles.pop(b)
                for t in range(NT):
                    k = NT * b + t
                    last = k == NCOL - 1
                    eps = ps_e.tile([P, DD], F32, tag="ep")
                    if last:
                        # bank-major emission: the (0,512) accumulation
                        # group finishes 5 matmuls early, so its tanh/mul/
                        # reduce overlap the (512,388) group's streams --
                        # shortens the serial end-of-kernel chain
                        for (no, nn) in NSP:
                            for c in range(5):
                                kp = P if c < 4 else KA
                                nc.tensor.matmul(
                                    eps[:, no:no + nn],
                                    cm[c][0:kp, t * P:(t + 1) * P],
                                    rhs[c][:, no:no + nn],
                                    start=(c == 0), stop=(c == 4),
                                )
                        z = zp.tile([P, DD], F32, tag="z")
                        junk = jp.tile([P, DD], F32, tag="junk")
                        for i, (no, nn) in enumerate(NSP):
                            nc.scalar.activation(z[:, no:no + nn],
                                                 eps[:, no:no + nn], AF.Tanh)
                            nc.vector.tensor_mul(junk[:, no:no + nn],
                                                 z[:, no:no + nn],
                                                 v_rep[:, no:no + nn])
                            nc.vector.tensor_reduce(
                                out=sc_ab[:, i:i + 1],
                                in_=junk[:, no:no + nn],
                                axis=AX.X, op=ALU.add,
                            )
                        nc.vector.tensor_reduce(
                            out=scores[b][:, t:t + 1], in_=sc_ab[:, :],
                            axis=AX.X, op=ALU.add,
                        )
                        continue
                    for c in range(5):
                        kp = P if c < 4 else KA
                        lhs = cm[c][0:kp, t * P:(t + 1) * P]
                        for (no, nn) in NSP:
                            nc.tensor.matmul(
                                eps[:, no:no + nn],
                                lhs,
                                rhs[c][:, no:no + nn],
                                start=(c == 0), stop=(c == 4),
                            )
                    z = zp.tile([P, DD], F32, tag="z")
                    nc.scalar.activation(z[:, :], eps[:, :], AF.Tanh)
                    junk = jp.tile([P, DD], F32, tag="junk")
                    nc.vector.tensor_mul(junk[:, :], z[:, :], v_rep[:, :])
                    if t % 2 == 1:
                        dump = jp.tile([P, DD], F32, tag="dump")
                        nc.scalar.activation(
                            dump[:, :], junk[:, :], AF.Copy,
                            accum_out=scores[b][:, t:t + 1],
                        )
                    else:
                        nc.vector.tensor_reduce(
                            out=scores[b][:, t:t + 1], in_=junk[:, :],
                            axis=AX.X, op=ALU.add,
                        )
                    # previous batch's 16 cols are long reduced by tile 6
                    # -> the PE transpose never stalls the queue
                    if b >= 1 and t == 6:
                        softmax_a(b - 1)
                    if b >= 1 and t == 12:
                        softmax_b(b - 1)

            softmax_a(BL - 1)
            softmax_b(BL - 1)
    return nc


_CACHE = {}


def _get_nc():
    if "nc" not in _CACHE:
        nc = build()
        nc.compile()
        _CACHE["nc"] = nc
    return _CACHE["nc"]


def make_in_maps(hidden, encoder_outputs, attn_W, attn_b, v):
    hidden = np.asarray(hidden, dtype=np.float32)
    attn_W = np.asarray(attn_W, dtype=np.float32)
    attn_b = np.asarray(attn_b, dtype=np.float32)
    v = np.asarray(v, dtype=np.float32).reshape(1, DD)
    enc = np.asarray(encoder_outputs, dtype=np.float32)

    WT = np.ascontiguousarray(attn_W.T)          # [1500, 900]
    rhsm = WT[DD:DD + 512].astype(NP_BF16)       # We^T rows 0:512
    we_tail = WT[DD + 512:IN_DIM]                # [88, 900] f32
    hb_all = hidden @ attn_W[:, :DD].T + attn_b  # [32, 900] f32 (exact)

    bones = np.ones((NT, NT), dtype=np.float32)

    in_maps = []
    for cidx in range(NCORES):
        bs = slice(cidx * BL, (cidx + 1) * BL)
        encT = enc[bs].reshape(SROWS, E2).T      # [600, 8192]
        enc4 = np.zeros((BL, KA, S), dtype=NP_BF16)
        for bb in range(BL):
            enc4[bb, :88] = encT[512:600, bb * S:(bb + 1) * S].astype(NP_BF16)
            enc4[bb, 88 + bb] = 1.0
        rhs4 = np.concatenate([we_tail, hb_all[bs]], axis=0)  # [92, 900]
        in_maps.append({
            "encm": np.ascontiguousarray(encT[:512]).astype(NP_BF16),
            "enc4": enc4.reshape(BL * KA, S),
            "rhsm": np.ascontiguousarray(rhsm),
            "rhs4": np.ascontiguousarray(rhs4.astype(NP_BF16)),
            "v": v,
            "ones": np.ones((1, P), dtype=np.float32),
            "bones": bones,
            "ident": np.eye(P, dtype=np.float32),
        })
    return in_maps


def run(in_maps, trace=False, **kw):
    nc = _get_nc()
    return run_bass_kernel_spmd(nc, in_maps, core_ids=list(range(NCORES)),
                                trace=trace, **kw)


def kernel(hidden, encoder_outputs, attn_W, attn_b, v):
    in_maps = make_in_maps(hidden, encoder_outputs, attn_W, attn_b, v)
    try:
        res = run(in_maps)
    except Exception:
        # transient device states (e.g. a previously wedged core) sometimes
        # clear on retry
        res = run(in_maps)
    out = np.concatenate([res.results[c]["out"] for c in range(NCORES)], axis=0)
    return np.ascontiguousarray(out, dtype=np.float32)


# revision 9
# speedup vs baseline: 1.0653x; 1.0653x over previous
"""Bahdanau-attention score kernel (softmax(v . tanh(W[h;enc]+b))) for 8 TRN2 cores.

v10: bf16 GEMM inputs (halves HBM traffic: enc 19.8MB->9.9MB/core, W
2.2->1.1MB; empirically 9.0e-3 max rel err vs the 2e-2 gate, dominated
by input quantization -- hb stays f32 on host, one-hot rows exact in
bf16), PE warm-up matmuls during the DMA prime window so the HAM clock
gate promotes to 2.4GHz before tile 0 (v9 lost ~10us to a 1.2GHz cold
window after an 8.8us DMA stall demoted it), 3-deep enc tile pool for
true 2-batch prefetch without burst stalls, per-batch softmax phase-1
(16 score cols each, emitted mid-next-batch so the in-order PE queue
never waits) instead of v9's two 32-col halves (the second of which
serialized the entire tail), and the v9 staples: host pre-transposed
enc, host-precomputed h_proj+b folded in via one-hot contraction rows,
DVE mul with the free-axis reduce alternating DVE/ACT, block-ones
matmul softmax denominator, bank-major emission on the final tile.

Self-contained: hardcodes shapes B=32, S=2048, ENC2=600, DD=900.
Sharding: data-parallel over batch (4 batches/core), weights replicated.
"""

import numpy as np
import ml_dtypes

import concourse.bass as bass  # noqa: F401
import concourse.mybir as mybir
import concourse.tile as tile
from concourse import bacc
from concourse.bass_utils import run_bass_kernel_spmd

F32 = mybir.dt.float32
F32R = mybir.dt.float32r
BF16 = mybir.dt.bfloat16
NP_BF16 = ml_dtypes.bfloat16
AF = mybir.ActivationFunctionType
ALU = mybir.AluOpType
AX = mybir.AxisListType

NCORES = 8
B, S, E2, DD = 32, 2048, 600, 900
IN_DIM = DD + E2            # 1500
BL = B // NCORES            # 4 batches per core
SROWS = BL * S              # 8192 s-rows per core
P = 128
NT = S // P                 # 16 s-tiles per batch
NCOL = SROWS // P           # 64 score columns
KA = 92                     # chunk-4 contraction: 88 e-rows + 4 one-hot rows
NSP = [(0, 512), (512, 388)]  # N splits of 900 (PSUM bank = 512 f32)
NWARM = 16                  # HAM warm-up matmuls during the DMA prime window


def build():
    nc = bacc.Bacc("TRN2", target_bir_lowering=False)
    encm_ext = nc.dram_tensor("encm", [512, SROWS], BF16, kind="ExternalInput")
    enc4_ext = nc.dram_tensor("enc4", [BL * KA, S], BF16, kind="ExternalInput")
    rhsm_ext = nc.dram_tensor("rhsm", [512, DD], BF16, kind="ExternalInput")
    rhs4_ext = nc.dram_tensor("rhs4", [KA, DD], BF16, kind="ExternalInput")
    v_ext = nc.dram_tensor("v", [1, DD], F32R, kind="ExternalInput")
    ones_ext = nc.dram_tensor("ones", [1, P], F32R, kind="ExternalInput")
    bones_ext = nc.dram_tensor("bones", [NT, NT], F32, kind="ExternalInput")
    ident_ext = nc.dram_tensor("ident", [P, P], F32, kind="ExternalInput")
    out_ext = nc.dram_tensor("out", [BL, S], F32, kind="ExternalOutput")

    with tile.TileContext(nc) as tc:
        with (
            tc.tile_pool(name="stat", bufs=1) as stat,
            tc.tile_pool(name="encp", bufs=3) as encp,
            tc.tile_pool(name="zp", bufs=4) as zp,
            tc.tile_pool(name="jp", bufs=3) as jp,
            tc.tile_pool(name="ps_e", bufs=4, space="PSUM") as ps_e,
        ):
            # tiny v/ones first (feed the PE warm-up block), then the
            # critical stream: rhs chunk c interleaved with batch-0 enc
            # chunk c (halves, so tile 0 gates on 5 half-tile DMAs not
            # 5 full ones).
            v_row = stat.tile([1, DD], F32R)
            nc.sync.dma_start(out=v_row[:, :], in_=v_ext.ap())
            ones_t = stat.tile([1, P], F32R)
            nc.sync.dma_start(out=ones_t[:, :], in_=ones_ext.ap())

            rhs = []
            cm_tiles = {}
            bones = stat.tile([NT, NT], F32)
            ident_f = stat.tile([P, P], F32)
            # Three parallel dispatch queues (Sync/Scalar/GpSimd HWDGE,
            # ~100-120GB/s each in the head) with the tile-0 critical
            # 2.5MB balanced across them in consumption order: chunk c's
            # weights + batch-0 first half land c-major so the PE can
            # accumulate as data arrives, second halves follow.
            for c in range(5):
                kp = P if c < 4 else KA
                rhs.append(stat.tile([kp, DD], BF16, name=f"rhs{c}"))
            b0 = [encp.tile([P if c < 4 else KA, S], BF16, tag=f"cm{c}",
                            name=f"cm{c}_0") for c in range(5)]

            def rhs_src(c):
                return (rhsm_ext.ap()[c * P:(c + 1) * P, :] if c < 4
                        else rhs4_ext.ap())

            def cm_src(c, b):
                return (encm_ext.ap()[c * P:(c + 1) * P, b * S:(b + 1) * S]
                        if c < 4 else enc4_ext.ap()[b * KA:(b + 1) * KA, :])

            H = 8 * P
            plan = {
                nc.sync: [("r", 0), ("h0", 1), ("r", 4), ("h1", 0),
                          ("h1", 3)],
                nc.scalar: [("h0", 0), ("r", 2), ("h0", 3), ("h1", 1),
                            ("h1", 4)],
                nc.gpsimd: [("r", 1), ("h0", 2), ("r", 3), ("h0", 4),
                            ("h1", 2)],
            }
            for eng, items in plan.items():
                for kind, c in items:
                    if kind == "r":
                        eng.dma_start(out=rhs[c][:, :], in_=rhs_src(c))
                    elif kind == "h0":
                        eng.dma_start(out=b0[c][:, 0:H],
                                      in_=cm_src(c, 0)[:, 0:H])
                    else:
                        eng.dma_start(out=b0[c][:, H:S],
                                      in_=cm_src(c, 0)[:, H:S])
            nc.scalar.dma_start(out=bones[:, :], in_=bones_ext.ap())
            nc.gpsimd.dma_start(out=ident_f[:, :], in_=ident_ext.ap())
            cm_tiles[0] = b0

            # v_rep = ones^T @ v, then NWARM warm-up matmuls chained as
            # one accumulation group into a scratch PSUM slot (chained so
            # DCE can't drop them; read once at the end): ~4us of PE busy
            # while the enc stream lands, so the HAM activity window
            # promotes the PE clock 4/8 -> 8/8 before tile 0.
            v_rep = stat.tile([P, DD], F32)
            psv = ps_e.tile([P, DD], F32, tag="ep", name="ps_vrep")
            for (no, nn) in NSP:
                nc.tensor.matmul(psv[:, no:no + nn], ones_t[0:1, :],
                                 v_row[0:1, no:no + nn],
                                 start=True, stop=True)
            nc.scalar.copy(v_rep[:, :], psv[:, :])
            psw = ps_e.tile([P, DD], F32, tag="ep", name="ps_warm")
            for i in range(NWARM):
                nc.tensor.matmul(psw[:, 0:512], ones_t[0:1, :],
                                 v_row[0:1, 0:512],
                                 start=(i == 0), stop=(i == NWARM - 1))

            def issue_batch(b):
                tiles = []
                for c in range(5):
                    kp = P if c < 4 else KA
                    t_ = encp.tile([kp, S], BF16, tag=f"cm{c}", name=f"cm{c}_{b}")
                    nc.sync.dma_start(out=t_[:, :], in_=cm_src(c, b))
                    tiles.append(t_)
                cm_tiles[b] = tiles

            issue_batch(1)

            scores = [stat.tile([P, NT], F32, name=f"scores{h}")
                      for h in range(BL)]
            sc_ab = stat.tile([P, 2], F32)
            e1 = [stat.tile([NT, P], F32, name=f"e1_{h}") for h in range(BL)]
            rs = [stat.tile([NT, 1], F32, name=f"rs{h}") for h in range(BL)]
            rfac = [stat.tile([NT, 1], F32, name=f"rfac{h}") for h in range(BL)]
            outf = [stat.tile([NT, P], F32, name=f"outf{h}") for h in range(BL)]
            dve_scr = stat.tile([1, 4], F32)

            # engine primes: absorb DMA sems before the hot loop; the
            # psw read also keeps the warm-up chain live through DCE
            nc.vector.tensor_copy(out=dve_scr[0:1, 0:1], in_=v_rep[0:1, 0:1])
            nc.vector.tensor_copy(out=dve_scr[0:1, 1:2], in_=bones[0:1, 0:1])
            nc.vector.tensor_copy(out=dve_scr[0:1, 2:3], in_=psw[0:1, 0:1])

            # ---------------- main loop ----------------
            def softmax_a(h):
                # transpose + exp one batch's 16 scores columns; the
                # per-batch scores tile was last written a full batch ago
                # so the PE transpose never waits
                pss = ps_e.tile([P, DD], F32, tag="ep", name=f"ps_sm{h}")
                nc.tensor.transpose(pss[0:NT, 0:P],
                                    scores[h][:, :],
                                    ident_f[:, :])
                nc.scalar.activation(
                    e1[h][:, :], pss[0:NT, 0:P], AF.Exp,
                    accum_out=rs[h][:, :],
                )

            def softmax_b(h):
                # denominator + scale + out DMA; emitted several tiles
                # after softmax_a so the psd matmul finds rs ready and
                # never blocks the in-order PE queue
                # bones is all-ones: psd = sum(rs[h]) replicated over 16
                # partitions
                psd = ps_e.tile([P, DD], F32, tag="ep", name=f"ps_bs{h}")
                nc.tensor.matmul(psd[0:NT, 0:1], bones[0:NT, 0:NT],
                                 rs[h][:, :], start=True, stop=True)
                nc.vector.reciprocal(rfac[h][:, :], psd[0:NT, 0:1])
                nc.vector.tensor_scalar_mul(outf[h][:, :], e1[h][:, :],
                                            rfac[h][:, 0:1])
                nc.sync.dma_start(
                    out=out_ext.ap()[h:h + 1, :].rearrange(
                        "b (t p) -> (b t) p", p=P),
                    in_=outf[h][:, :],
                )

            for b in range(BL):
                if b + 2 < BL:
                    issue_batch(b + 2)
                cm = cm_tiles.pop(b)
                for t in range(NT):
                    k = NT * b + t
                    last = k == NCOL - 1
                    eps = ps_e.tile([P, DD], F32, tag="ep")
                    if last:
                        # bank-major emission: the (0,512) accumulation
                        # group finishes 5 matmuls early, so its tanh/mul/
                        # reduce overlap the (512,388) group's streams --
                        # shortens the serial end-of-kernel chain
                        for (no, nn) in NSP:
                            for c in range(5):
                                kp = P if c < 4 else KA
                                nc.tensor.matmul(
                                    eps[:, no:no + nn],
                                    cm[c][0:kp, t * P:(t + 1) * P],
                                    rhs[c][:, no:no + nn],
                                    start=(c == 0), stop=(c == 4),
                                )
                        z = zp.tile([P, DD], F32, tag="z")
                        junk = jp.tile([P, DD], F32, tag="junk")
                        for i, (no, nn) in enumerate(NSP):
                            nc.scalar.activation(z[:, no:no + nn],
                                                 eps[:, no:no + nn], AF.Tanh)
                            nc.vector.tensor_mul(junk[:, no:no + nn],
                                                 z[:, no:no + nn],
                                                 v_rep[:, no:no + nn])
                            nc.vector.tensor_reduce(
                                out=sc_ab[:, i:i + 1],
                                in_=junk[:, no:no + nn],
                                axis=AX.X, op=ALU.add,
                            )
                        nc.vector.tensor_reduce(
                            out=scores[b][:, t:t + 1], in_=sc_ab[:, :],
                            axis=AX.X, op=ALU.add,
                        )
                        continue
                    for c in range(5):
                        kp = P if c < 4 else KA
                        lhs = cm[c][0:kp, t * P:(t + 1) * P]
                        for (no, nn) in NSP:
                            nc.tensor.matmul(
                                eps[:, no:no + nn],
                                lhs,
                                rhs[c][:, no:no + nn],
                                start=(c == 0), stop=(c == 4),
                            )
                    z = zp.tile([P, DD], F32, tag="z")
                    nc.scalar.activation(z[:, :], eps[:, :], AF.Tanh)
                    junk = jp.tile([P, DD], F32, tag="junk")
                    nc.vector.tensor_mul(junk[:, :], z[:, :], v_rep[:, :])
                    if t % 2 == 1:
                        dump = jp.tile([P, DD], F32, tag="dump")
                        nc.scalar.activation(
                            dump[:, :], junk[:, :], AF.Copy,
                            accum_out=scores[b][:, t:t + 1],
                        )
                    else:
                        nc.vector.tensor_reduce(
                            out=scores[b][:, t:t + 1], in_=junk[:, :],
                            axis=AX.X, op=ALU.add,
                        )
                    # previous batch's 16 cols are long reduced by tile 6
                    # -> the PE transpose never stalls the queue
                    if b >= 1 and t == 6:
                        softmax_a(b - 1)
                    if b >= 1 and t == 12:
                        softmax_b(b - 1)

            softmax_a(BL - 1)
            softmax_b(BL - 1)
    return nc


_CACHE = {}


def _get_nc():
    if "nc" not in _CACHE:
        nc = build()
        nc.compile()
        _CACHE["nc"] = nc
    return _CACHE["nc"]


def make_in_maps(hidden, encoder_outputs, attn_W, attn_b, v):
    hidden = np.asarray(hidden, dtype=np.float32)
    attn_W = np.asarray(attn_W, dtype=np.float32)
    attn_b = np.asarray(attn_b, dtype=np.float32)
    v = np.asarray(v, dtype=np.float32).reshape(1, DD)
    enc = np.asarray(encoder_outputs, dtype=np.float32)

    WT = np.ascontiguousarray(attn_W.T)          # [1500, 900]
    rhsm = WT[DD:DD + 512].astype(NP_BF16)       # We^T rows 0:512
    we_tail = WT[DD + 512:IN_DIM]                # [88, 900] f32
    hb_all = hidden @ attn_W[:, :DD].T + attn_b  # [32, 900] f32 (exact)

    bones = np.ones((NT, NT), dtype=np.float32)

    in_maps = []
    for cidx in range(NCORES):
        bs = slice(cidx * BL, (cidx + 1) * BL)
        encT = enc[bs].reshape(SROWS, E2).T      # [600, 8192]
        enc4 = np.zeros((BL, KA, S), dtype=NP_BF16)
        for bb in range(BL):
            enc4[bb, :88] = encT[512:600, bb * S:(bb + 1) * S].astype(NP_BF16)
            enc4[bb, 88 + bb] = 1.0
        rhs4 = np.concatenate([we_tail, hb_all[bs]], axis=0)  # [92, 900]
        in_maps.append({
            "encm": np.ascontiguousarray(encT[:512]).astype(NP_BF16),
            "enc4": enc4.reshape(BL * KA, S),
            "rhsm": np.ascontiguousarray(rhsm),
            "rhs4": np.ascontiguousarray(rhs4.astype(NP_BF16)),
            "v": v,
            "ones": np.ones((1, P), dtype=np.float32),
            "bones": bones,
            "ident": np.eye(P, dtype=np.float32),
        })
    return in_maps


def run(in_maps, trace=False, **kw):
    nc = _get_nc()
    return run_bass_kernel_spmd(nc, in_maps, core_ids=list(range(NCORES)),
                                trace=trace, **kw)


def kernel(hidden, encoder_outputs, attn_W, attn_b, v):
    in_maps = make_in_maps(hidden, encoder_outputs, attn_W, attn_b, v)
    try:
        res = run(in_maps)
    except Exception:
        # transient device states (e.g. a previously wedged core) sometimes
        # clear on retry
        res = run(in_maps)
    out = np.concatenate([res.results[c]["out"] for c in range(NCORES)], axis=0)
    return np.ascontiguousarray(out, dtype=np.float32)


# revision 11
# speedup vs baseline: 1.1045x; 1.0368x over previous
"""Bahdanau-attention score kernel (softmax(v . tanh(W[h;enc]+b))) for 8 TRN2 cores.

v10: bf16 GEMM inputs (halves HBM traffic: enc 19.8MB->9.9MB/core, W
2.2->1.1MB; empirically 9.0e-3 max rel err vs the 2e-2 gate, dominated
by input quantization -- hb stays f32 on host, one-hot rows exact in
bf16), PE warm-up matmuls during the DMA prime window so the HAM clock
gate promotes to 2.4GHz before tile 0 (v9 lost ~10us to a 1.2GHz cold
window after an 8.8us DMA stall demoted it), 3-deep enc tile pool for
true 2-batch prefetch without burst stalls, per-batch softmax phase-1
(16 score cols each, emitted mid-next-batch so the in-order PE queue
never waits) instead of v9's two 32-col halves (the second of which
serialized the entire tail), and the v9 staples: host pre-transposed
enc, host-precomputed h_proj+b folded in via one-hot contraction rows,
DVE mul with the free-axis reduce alternating DVE/ACT, block-ones
matmul softmax denominator, bank-major emission on the final tile.

Self-contained: hardcodes shapes B=32, S=2048, ENC2=600, DD=900.
Sharding: data-parallel over batch (4 batches/core), weights replicated.
"""

import numpy as np
import ml_dtypes

import concourse.bass as bass  # noqa: F401
import concourse.mybir as mybir
import concourse.tile as tile
from concourse import bacc
from concourse.bass_utils import run_bass_kernel_spmd

F32 = mybir.dt.float32
F32R = mybir.dt.float32r
BF16 = mybir.dt.bfloat16
NP_BF16 = ml_dtypes.bfloat16
AF = mybir.ActivationFunctionType
ALU = mybir.AluOpType
AX = mybir.AxisListType

NCORES = 8
B, S, E2, DD = 32, 2048, 600, 900
IN_DIM = DD + E2            # 1500
BL = B // NCORES            # 4 batches per core
SROWS = BL * S              # 8192 s-rows per core
P = 128
NT = S // P                 # 16 s-tiles per batch
NCOL = SROWS // P           # 64 score columns
KA = 92                     # chunk-4 contraction: 88 e-rows + 4 one-hot rows
NSP = [(0, 512), (512, 388)]  # N splits of 900 (PSUM bank = 512 f32)
NWARM = 8                   # HAM warm-up matmuls during the DMA prime window


def build():
    nc = bacc.Bacc("TRN2", target_bir_lowering=False)
    encm_ext = nc.dram_tensor("encm", [512, SROWS], BF16, kind="ExternalInput")
    enc4_ext = nc.dram_tensor("enc4", [BL * KA, S], BF16, kind="ExternalInput")
    rhsm_ext = nc.dram_tensor("rhsm", [512, DD], BF16, kind="ExternalInput")
    rhs4_ext = nc.dram_tensor("rhs4", [KA, DD], BF16, kind="ExternalInput")
    v_ext = nc.dram_tensor("v", [1, DD], F32R, kind="ExternalInput")
    ones_ext = nc.dram_tensor("ones", [1, P], F32R, kind="ExternalInput")
    bones_ext = nc.dram_tensor("bones", [NT, NT], F32, kind="ExternalInput")
    ident_ext = nc.dram_tensor("ident", [P, P], F32, kind="ExternalInput")
    out_ext = nc.dram_tensor("out", [BL, S], F32, kind="ExternalOutput")

    with tile.TileContext(nc) as tc:
        with (
            tc.tile_pool(name="stat", bufs=1) as stat,
            tc.tile_pool(name="encp", bufs=3) as encp,
            tc.tile_pool(name="zp", bufs=4) as zp,
            tc.tile_pool(name="jp", bufs=3) as jp,
            tc.tile_pool(name="ps_e", bufs=4, space="PSUM") as ps_e,
        ):
            # tiny v/ones first (feed the PE warm-up block), then the
            # critical stream: rhs chunk c interleaved with batch-0 enc
            # chunk c (halves, so tile 0 gates on 5 half-tile DMAs not
            # 5 full ones).
            v_row = stat.tile([1, DD], F32R)
            nc.sync.dma_start(out=v_row[:, :], in_=v_ext.ap())
            ones_t = stat.tile([1, P], F32R)
            nc.sync.dma_start(out=ones_t[:, :], in_=ones_ext.ap())

            rhs = []
            cm_tiles = {}
            bones = stat.tile([NT, NT], F32)
            ident_f = stat.tile([P, P], F32)
            # Three parallel dispatch queues (Sync/Scalar/GpSimd HWDGE,
            # ~100-120GB/s each in the head) with the tile-0 critical
            # 2.5MB balanced across them in consumption order: chunk c's
            # weights + batch-0 first half land c-major so the PE can
            # accumulate as data arrives, second halves follow.
            for c in range(5):
                kp = P if c < 4 else KA
                rhs.append(stat.tile([kp, DD], BF16, name=f"rhs{c}"))
            b0 = [encp.tile([P if c < 4 else KA, S], BF16, tag=f"cm{c}",
                            name=f"cm{c}_0") for c in range(5)]

            def rhs_src(c):
                return (rhsm_ext.ap()[c * P:(c + 1) * P, :] if c < 4
                        else rhs4_ext.ap())

            def cm_src(c, b):
                return (encm_ext.ap()[c * P:(c + 1) * P, b * S:(b + 1) * S]
                        if c < 4 else enc4_ext.ap()[b * KA:(b + 1) * KA, :])

            H = 8 * P
            # Single HWDGE engine serves all dispatch queues round-robin,
            # so parallel queues add no bandwidth -- one priority-ordered
            # Sync stream wins: chunk c's weights + batch-0 first half
            # land c-major so the PE accumulates tile 0 as data arrives;
            # second halves (tiles 8-15) follow.
            for c in range(5):
                nc.sync.dma_start(out=rhs[c][:, :], in_=rhs_src(c))
                nc.sync.dma_start(out=b0[c][:, 0:H], in_=cm_src(c, 0)[:, 0:H])
            for c in range(5):
                nc.sync.dma_start(out=b0[c][:, H:S], in_=cm_src(c, 0)[:, H:S])
            nc.sync.dma_start(out=bones[:, :], in_=bones_ext.ap())
            nc.sync.dma_start(out=ident_f[:, :], in_=ident_ext.ap())
            cm_tiles[0] = b0

            # v_rep = ones^T @ v, then NWARM warm-up matmuls chained as
            # one accumulation group into a scratch PSUM slot (chained so
            # DCE can't drop them; read once at the end): ~4us of PE busy
            # while the enc stream lands, so the HAM activity window
            # promotes the PE clock 4/8 -> 8/8 before tile 0.
            v_rep = stat.tile([P, DD], F32)
            psv = ps_e.tile([P, DD], F32, tag="ep", name="ps_vrep")
            for (no, nn) in NSP:
                nc.tensor.matmul(psv[:, no:no + nn], ones_t[0:1, :],
                                 v_row[0:1, no:no + nn],
                                 start=True, stop=True)
            nc.scalar.copy(v_rep[:, :], psv[:, :])
            psw = ps_e.tile([P, DD], F32, tag="ep", name="ps_warm")
            for i in range(NWARM):
                nc.tensor.matmul(psw[:, 0:512], ones_t[0:1, :],
                                 v_row[0:1, 0:512],
                                 start=(i == 0), stop=(i == NWARM - 1))

            def issue_batch(b):
                tiles = []
                for c in range(5):
                    kp = P if c < 4 else KA
                    t_ = encp.tile([kp, S], BF16, tag=f"cm{c}", name=f"cm{c}_{b}")
                    nc.sync.dma_start(out=t_[:, :], in_=cm_src(c, b))
                    tiles.append(t_)
                cm_tiles[b] = tiles

            issue_batch(1)

            scores = [stat.tile([P, NT], F32, name=f"scores{h}")
                      for h in range(BL)]
            sc_ab = stat.tile([P, 2], F32)
            e1 = [stat.tile([NT, P], F32, name=f"e1_{h}") for h in range(BL)]
            rs = [stat.tile([NT, 1], F32, name=f"rs{h}") for h in range(BL)]
            rfac = [stat.tile([NT, 1], F32, name=f"rfac{h}") for h in range(BL)]
            outf = [stat.tile([NT, P], F32, name=f"outf{h}") for h in range(BL)]
            dve_scr = stat.tile([1, 4], F32)
            qwake = stat.tile([1, 1], F32R)

            # engine primes: absorb DMA sems before the hot loop; the
            # psw read also keeps the warm-up chain live through DCE
            nc.vector.tensor_copy(out=dve_scr[0:1, 0:1], in_=v_rep[0:1, 0:1])
            nc.vector.tensor_copy(out=dve_scr[0:1, 1:2], in_=bones[0:1, 0:1])
            nc.vector.tensor_copy(out=dve_scr[0:1, 2:3], in_=psw[0:1, 0:1])

            # ---------------- main loop ----------------
            def softmax_a(h):
                # transpose + exp one batch's 16 scores columns; the
                # per-batch scores tile was last written a full batch ago
                # so the PE transpose never waits
                pss = ps_e.tile([P, DD], F32, tag="ep", name=f"ps_sm{h}")
                nc.tensor.transpose(pss[0:NT, 0:P],
                                    scores[h][:, :],
                                    ident_f[:, :])
                nc.scalar.activation(
                    e1[h][:, :], pss[0:NT, 0:P], AF.Exp,
                    accum_out=rs[h][:, :],
                )

            def softmax_b(h):
                # denominator + scale + out DMA; emitted several tiles
                # after softmax_a so the psd matmul finds rs ready and
                # never blocks the in-order PE queue
                # bones is all-ones: psd = sum(rs[h]) replicated over 16
                # partitions
                psd = ps_e.tile([P, DD], F32, tag="ep", name=f"ps_bs{h}")
                nc.tensor.matmul(psd[0:NT, 0:1], bones[0:NT, 0:NT],
                                 rs[h][:, :], start=True, stop=True)
                nc.vector.reciprocal(rfac[h][:, :], psd[0:NT, 0:1])
                nc.vector.tensor_scalar_mul(outf[h][:, :], e1[h][:, :],
                                            rfac[h][:, 0:1])
                nc.sync.dma_start(
                    out=out_ext.ap()[h:h + 1, :].rearrange(
                        "b (t p) -> (b t) p", p=P),
                    in_=outf[h][:, :],
                )

            for b in range(BL):
                if b + 2 < BL:
                    issue_batch(b + 2)
                cm = cm_tiles.pop(b)
                for t in range(NT):
                    k = NT * b + t
                    last = k == NCOL - 1
                    eps = ps_e.tile([P, DD], F32, tag="ep")
                    if last:
                        # bank-major emission: the (0,512) accumulation
                        # group finishes 5 matmuls early, so its tanh/mul/
                        # reduce overlap the (512,388) group's streams --
                        # shortens the serial end-of-kernel chain
                        for (no, nn) in NSP:
                            for c in range(5):
                                kp = P if c < 4 else KA
                                nc.tensor.matmul(
                                    eps[:, no:no + nn],
                                    cm[c][0:kp, t * P:(t + 1) * P],
                                    rhs[c][:, no:no + nn],
                                    start=(c == 0), stop=(c == 4),
                                )
                        # tail chain: bank A reduces on DVE, bank B on
                        # ACT accum-copy, so the two halves' post-work
                        # drains in parallel instead of serializing on DVE
                        z = zp.tile([P, DD], F32, tag="z")
                        junk = jp.tile([P, DD], F32, tag="junk")
                        dump = jp.tile([P, DD], F32, tag="dump")
                        (no0, nn0), (no1, nn1) = NSP
                        nc.scalar.activation(z[:, no0:no0 + nn0],
                                             eps[:, no0:no0 + nn0], AF.Tanh)
                        nc.vector.tensor_mul(junk[:, no0:no0 + nn0],
                                             z[:, no0:no0 + nn0],
                                             v_rep[:, no0:no0 + nn0])
                        nc.vector.tensor_reduce(
                            out=sc_ab[:, 0:1], in_=junk[:, no0:no0 + nn0],
                            axis=AX.X, op=ALU.add,
                        )
                        nc.scalar.activation(z[:, no1:no1 + nn1],
                                             eps[:, no1:no1 + nn1], AF.Tanh)
                        nc.vector.tensor_mul(junk[:, no1:no1 + nn1],
                                             z[:, no1:no1 + nn1],
                                             v_rep[:, no1:no1 + nn1])
                        nc.scalar.activation(
                            dump[:, no1:no1 + nn1], junk[:, no1:no1 + nn1],
                            AF.Copy, accum_out=sc_ab[:, 1:2],
                        )
                        nc.vector.tensor_reduce(
                            out=scores[b][:, t:t + 1], in_=sc_ab[:, :],
                            axis=AX.X, op=ALU.add,
                        )
                        continue
                    for c in range(5):
                        kp = P if c < 4 else KA
                        lhs = cm[c][0:kp, t * P:(t + 1) * P]
                        for (no, nn) in NSP:
                            nc.tensor.matmul(
                                eps[:, no:no + nn],
                                lhs,
                                rhs[c][:, no:no + nn],
                                start=(c == 0), stop=(c == 4),
                            )
                    z = zp.tile([P, DD], F32, tag="z")
                    nc.scalar.activation(z[:, :], eps[:, :], AF.Tanh)
                    junk = jp.tile([P, DD], F32, tag="junk")
                    nc.vector.tensor_mul(junk[:, :], z[:, :], v_rep[:, :])
                    if t % 2 == 1:
                        dump = jp.tile([P, DD], F32, tag="dump")
                        nc.scalar.activation(
                            dump[:, :], junk[:, :], AF.Copy,
                            accum_out=scores[b][:, t:t + 1],
                        )
                    else:
                        nc.vector.tensor_reduce(
                            out=scores[b][:, t:t + 1], in_=junk[:, :],
                            axis=AX.X, op=ALU.add,
                        )
                    # previous batch's 16 cols are long reduced by tile 6
                    # -> the PE transpose never stalls the queue
                    if b >= 1 and t == 6:
                        softmax_a(b - 1)
                    if b >= 1 and t == 12:
                        softmax_b(b - 1)
                    if b == BL - 1 and t == 13:
                        # wake the idle Sync DMA queue so the final 8KB
                        # out DMA doesn't pay ~1.5us cold-queue latency
                        nc.sync.dma_start(out=qwake[0:1, 0:1],
                                          in_=ones_ext.ap()[0:1, 0:1])

            softmax_a(BL - 1)
            softmax_b(BL - 1)
    return nc


_CACHE = {}


def _get_nc():
    if "nc" not in _CACHE:
        nc = build()
        nc.compile()
        _CACHE["nc"] = nc
    return _CACHE["nc"]


def make_in_maps(hidden, encoder_outputs, attn_W, attn_b, v):
    hidden = np.asarray(hidden, dtype=np.float32)
    attn_W = np.asarray(attn_W, dtype=np.float32)
    attn_b = np.asarray(attn_b, dtype=np.float32)
    v = np.asarray(v, dtype=np.float32).reshape(1, DD)
    enc = np.asarray(encoder_outputs, dtype=np.float32)

    WT = np.ascontiguousarray(attn_W.T)          # [1500, 900]
    rhsm = WT[DD:DD + 512].astype(NP_BF16)       # We^T rows 0:512
    we_tail = WT[DD + 512:IN_DIM]                # [88, 900] f32
    hb_all = hidden @ attn_W[:, :DD].T + attn_b  # [32, 900] f32 (exact)

    bones = np.ones((NT, NT), dtype=np.float32)

    in_maps = []
    for cidx in range(NCORES):
        bs = slice(cidx * BL, (cidx + 1) * BL)
        encT = enc[bs].reshape(SROWS, E2).T      # [600, 8192]
        enc4 = np.zeros((BL, KA, S), dtype=NP_BF16)
        for bb in range(BL):
            enc4[bb, :88] = encT[512:600, bb * S:(bb + 1) * S].astype(NP_BF16)
            enc4[bb, 88 + bb] = 1.0
        rhs4 = np.concatenate([we_tail, hb_all[bs]], axis=0)  # [92, 900]
        in_maps.append({
            "encm": np.ascontiguousarray(encT[:512]).astype(NP_BF16),
            "enc4": enc4.reshape(BL * KA, S),
            "rhsm": np.ascontiguousarray(rhsm),
            "rhs4": np.ascontiguousarray(rhs4.astype(NP_BF16)),
            "v": v,
            "ones": np.ones((1, P), dtype=np.float32),
            "bones": bones,
            "ident": np.eye(P, dtype=np.float32),
        })
    return in_maps


def run(in_maps, trace=False, **kw):
    nc = _get_nc()
    return run_bass_kernel_spmd(nc, in_maps, core_ids=list(range(NCORES)),
                                trace=trace, **kw)


def kernel(hidden, encoder_outputs, attn_W, attn_b, v):
    in_maps = make_in_maps(hidden, encoder_outputs, attn_W, attn_b, v)
    try:
        res = run(in_maps)
    except Exception:
        # transient device states (e.g. a previously wedged core) sometimes
        # clear on retry
        res = run(in_maps)
    out = np.concatenate([res.results[c]["out"] for c in range(NCORES)], axis=0)
    return np.ascontiguousarray(out, dtype=np.float32)
